# revision 1
# baseline (speedup 1.0000x reference)
"""Contextual LSTM cell on 8 Trainium2 NeuronCores.

Strategy:
  - Shard the batch dim (B=65536) across 8 cores (8192 each), replicate weights.
  - All 15 gate matmuls fused into ONE (1024 x 832) @ (832 x B) matmul:
        rows:  [gate_i | gate_f | gate_c | gate_o]      (4 x 256)
        cols:  [x (256) | h (256) | c (256) | topic (64)]
    with -w_ho folded in and the (gate_c, c) block identically zero (skipped).
  - Matmul in fp16 (1 cycle/row on PE vs 4 for fp32), accumulated fp32 in PSUM.
    x/h/topic + weights are cast to fp16 on the host; c is shipped fp32 (needed
    exactly for cc = cf*c + ...) and cast to fp16 on-device for the matmul.
  - Per-gate bias is fused into the ScalarE activation (sigmoid/tanh) that
    reads the PSUM bank directly; cc/ch elementwise runs fp32 on VectorE.
"""

import os
import numpy as np

import concourse.bass as bass
import concourse.bacc as bacc
import concourse.mybir as mybir
from concourse.tile import TileContext
from concourse.bass_utils import run_bass_kernel_spmd

I, H, T, B = 256, 256, 64, 65536
NCORES = 8
BS = B // NCORES          # 8192 batch columns per core
NT = 512                  # columns per chunk (one PSUM bank of fp32)
NCHUNK = BS // NT         # 16
KB = 7                    # k-blocks: x:2, h:2, c:2, topic:1(64 rows)
MB = 8                    # m-blocks: 4 gates x 2

FP16 = mybir.dt.float16
FP32 = mybir.dt.float32
SIG = mybir.ActivationFunctionType.Sigmoid
TANH = mybir.ActivationFunctionType.Tanh

_PROGRAM = None
_LAST_RESULTS = None  # for test harness introspection


def _build_program(repeat=1):
    # Bacc (not Bass): its compile() pass splits multi-semaphore waits into
    # InstEventSemaphore preludes — walrus rejects >1 sync wait per
    # instruction otherwise.
    nc = bacc.Bacc()

    xh = nc.declare_dram_parameter("xh", [2 * H, BS], FP16, isOutput=False)
    c_in = nc.declare_dram_parameter("c", [H, BS], FP32, isOutput=False)
    topic = nc.declare_dram_parameter("topic", [T, BS], FP16, isOutput=False)
    wt = nc.declare_dram_parameter("wt", [128, KB * 1024], FP16, isOutput=False)
    biases = nc.declare_dram_parameter("biases", [128, MB], FP32, isOutput=False)
    ch_out = nc.declare_dram_parameter("ch", [H, BS], FP32, isOutput=True)
    cc_out = nc.declare_dram_parameter("cc", [H, BS], FP32, isOutput=True)

    chunks = [(i * NT, NT) for i in range(NCHUNK)]

    with TileContext(nc) as tc:
        with (
            tc.tile_pool(name="const", bufs=1) as constp,
            tc.tile_pool(name="zin", bufs=4) as zp,
            tc.tile_pool(name="gates", bufs=2) as gp,
            tc.tile_pool(name="psum", bufs=1, space="PSUM") as pp,
        ):
            wsb = constp.tile([128, KB * 1024], FP16, tag="w", name="wsb")
            bsb = constp.tile([128, MB], FP32, tag="b", name="bsb")
            # k0 weights first: the very first matmul needs only this block.
            # Remaining weight blocks + biases are interleaved between the
            # first chunk's input DMAs below (8 HWDGE queues run them in
            # parallel) so the PE can start ~2us after kernel entry.
            nc.sync.dma_start(out=wsb[:, 0:1024], in_=wt[:, 0:1024])
            pending_w = list(range(1, KB)) + [None]  # None -> bias DMA

            def weight_dma():
                if not pending_w:
                    return
                k = pending_w.pop(0)
                if k is None:
                    nc.sync.dma_start(out=bsb[:], in_=biases[:])
                else:
                    nc.sync.dma_start(out=wsb[:, k * 1024:(k + 1) * 1024],
                                      in_=wt[:, k * 1024:(k + 1) * 1024])

            # PE warm-up: ~2.5us of tiny matmuls hidden under the initial DMA
            # fill releases the HAM clock gate (cold PE runs at 1.2 GHz for
            # its first ~3.4us of activity) before the real stream begins.
            wz = constp.tile([128, 64], FP16, tag="wz", name="wz")
            nc.vector.memset(wz[:], 0.0)
            pdum = pp.tile([128, NT], FP32, tag="ps0", name="pdum")
            for _ in range(28):
                nc.tensor.matmul(pdum[0:64, 0:64], wz[:, 0:64], wz[:, 0:64],
                                 start=True, stop=True)

            for rn in range(repeat * len(chunks)):
                r, n = divmod(rn, len(chunks))
                c0, nt = chunks[n]
                c1 = c0 + nt

                # ---- load inputs for this column chunk ----
                # batched 3D-AP DMAs: one transfer for all four 128-row
                # blocks of [x; h], one for both blocks of c
                z = []
                for j in range(4):
                    zj = zp.tile([128, nt], FP16, tag=f"z{j}", name=f"z{j}_{n}")
                    nc.sync.dma_start(out=zj[:], in_=xh[j * 128:(j + 1) * 128, c0:c1])
                    if rn == 0:
                        weight_dma()
                    z.append(zj)
                cf32 = []
                for j in range(2):
                    cj = zp.tile([128, nt], FP32, tag=f"c{j}", name=f"c{j}_{n}")
                    nc.sync.dma_start(out=cj[:], in_=c_in[j * 128:(j + 1) * 128, c0:c1])
                    if rn == 0:
                        weight_dma()
                    c16 = zp.tile([128, nt], FP16, tag=f"c16_{j}", name=f"c16_{j}_{n}")
                    nc.vector.tensor_copy(out=c16[:], in_=cj[:])
                    cf32.append(cj[:])
                    z.append(c16)
                # topic duplicated into partitions 64-127 so the two topic
                # matmuls of an m-pair can run concurrently via row packing
                tp = zp.tile([128, nt], FP16, tag="tp", name=f"tp_{n}")
                nc.sync.dma_start(out=tp[0:T, :], in_=topic[:, c0:c1])
                if rn == 0:
                    weight_dma()
                nc.sync.dma_start(out=tp[T:128, :], in_=topic[:, c0:c1])
                while rn == 0 and pending_w:
                    weight_dma()

                # ---- the fused gate matmul ----
                # Per m-pair: all K=128 blocks of both m's first, then the two
                # K=64 topic matmuls back-to-back on distinct row-groups
                # ((0,0) and (64,0)) — they execute concurrently in the PE
                # array, halving the topic block's cost.
                ps = [None] * MB
                for m0 in range(0, MB, 2):
                    for m in (m0, m0 + 1):
                        pst = pp.tile([128, nt], FP32, tag=f"ps{m}", name=f"ps{m}_{n}")
                        # gate_c (m 4,5) has no c-term: skip the zero blocks
                        ks = [k for k in range(6) if not (m in (4, 5) and k in (4, 5))]
                        for i, k in enumerate(ks):
                            lhsT = wsb[:, k * 1024 + m * 128: k * 1024 + (m + 1) * 128]
                            nc.tensor.matmul(
                                pst[:], lhsT, z[k][:],
                                start=(i == 0), stop=False,
                            )
                        ps[m] = pst
                    for m in (m0, m0 + 1):
                        p0 = T * (m & 1)
                        lhsT = wsb[p0:p0 + T, 6 * 1024 + m * 128: 6 * 1024 + (m + 1) * 128]
                        nc.tensor.matmul(
                            ps[m][:], lhsT, tp[p0:p0 + T, :],
                            start=False, stop=True,
                            tile_position=(p0, 0),
                        )

                # ---- gate activations (bias fused, reads PSUM) ----
                def act(m, fn, nm):
                    t = gp.tile([128, nt], FP32, tag=nm, name=f"{nm}_{n}")
                    nc.scalar.activation(out=t[:], in_=ps[m][:], func=fn,
                                         bias=bsb[:, m:m + 1])
                    return t

                for half in range(2):
                    ci = act(0 + half, SIG, f"ci{half}")
                    cf = act(2 + half, SIG, f"cf{half}")
                    tg = act(4 + half, TANH, f"tg{half}")
                    co = act(6 + half, SIG, f"co{half}")

                    t1 = gp.tile([128, nt], FP32, tag=f"t1{half}", name=f"t1{half}_{n}")
                    nc.vector.tensor_mul(t1[:], ci[:], tg[:])
                    t2 = gp.tile([128, nt], FP32, tag=f"t2{half}", name=f"t2{half}_{n}")
                    nc.vector.tensor_mul(t2[:], cf[:], cf32[half])
                    cct = gp.tile([128, nt], FP32, tag=f"cc{half}", name=f"cc{half}_{n}")
                    nc.vector.tensor_add(cct[:], t1[:], t2[:])
                    tcc = gp.tile([128, nt], FP32, tag=f"tcc{half}", name=f"tcc{half}_{n}")
                    nc.scalar.activation(out=tcc[:], in_=cct[:], func=TANH)
                    cht = gp.tile([128, nt], FP32, tag=f"chh{half}", name=f"chh{half}_{n}")
                    nc.vector.tensor_mul(cht[:], co[:], tcc[:])
                    r0, r1 = half * 128, (half + 1) * 128
                    nc.sync.dma_start(out=cc_out[r0:r1, c0:c1], in_=cct[:])
                    nc.sync.dma_start(out=ch_out[r0:r1, c0:c1], in_=cht[:])

    nc.finalize()
    return nc


def _prep_weights(inp):
    """Assemble the fused (1024, 832) weight and return lhsT blocks + biases."""
    Wf = np.zeros((1024, 832), np.float32)

    def put(g, blocks):
        r = g * 256
        for j, wb in enumerate(blocks):
            if wb is None:
                continue
            col = j * 256
            Wf[r:r + 256, col:col + wb.shape[1]] = wb

    put(0, [inp["w_ii"], inp["w_hi"], inp["w_ci"], inp["w_bi"]])
    put(1, [inp["w_if"], inp["w_hf"], inp["w_cf"], inp["w_bf"]])
    put(2, [inp["w_ic"], inp["w_hc"], None, inp["w_bc"]])
    put(3, [inp["w_io"], -inp["w_ho"], inp["w_co"], inp["w_bo"]])

    wT = np.zeros((KB * 128, 1024), np.float32)
    wT[:832] = Wf.T
    # duplicate topic weight rows into partitions 64-127 of the k=6 block
    # (row-packed topic matmuls read them at base_partition 64)
    wT[832:896] = wT[768:832]
    # (7,128,1024) -> (128, 7*1024) with block-k contiguous in the free dim
    wt_host = np.ascontiguousarray(
        wT.reshape(KB, 128, 1024).transpose(1, 0, 2).reshape(128, KB * 1024)
    ).astype(np.float16)

    bias_all = np.concatenate(
        [inp["bias_i"], inp["bias_f"], inp["bias_c"], inp["bias_o"]], axis=0
    ).reshape(MB, 128)
    bias_host = np.ascontiguousarray(bias_all.T).astype(np.float32)
    return wt_host, bias_host


def kernel(**inputs):
    global _PROGRAM, _LAST_RESULTS
    if _PROGRAM is None:
        _PROGRAM = _build_program()
    nc = _PROGRAM

    inp = {k: np.asarray(v, dtype=np.float32) for k, v in inputs.items()}
    wt_host, bias_host = _prep_weights(inp)

    xh16 = np.concatenate([inp["x"], inp["h"]], axis=0).astype(np.float16)
    t16 = inp["topic"].astype(np.float16)
    c32 = inp["c"]

    in_maps = []
    for i in range(NCORES):
        sl = slice(i * BS, (i + 1) * BS)
        in_maps.append({
            "xh": np.ascontiguousarray(xh16[:, sl]),
            "c": np.ascontiguousarray(c32[:, sl]),
            "topic": np.ascontiguousarray(t16[:, sl]),
            "wt": wt_host,
            "biases": bias_host,
        })

    res = run_bass_kernel_spmd(
        nc, in_maps, list(range(NCORES)),
        trace=bool(os.environ.get("KERNEL_TRACE")),
    )
    _LAST_RESULTS = res

    ch = np.concatenate([res.results[i]["ch"] for i in range(NCORES)], axis=1)
    cc = np.concatenate([res.results[i]["cc"] for i in range(NCORES)], axis=1)
    return np.stack([ch, cc], axis=0)



# revision 3
# speedup vs baseline: 1.1693x; 1.1693x over previous
"""Contextual LSTM cell on 8 Trainium2 NeuronCores — fp8 DoubleRow edition.

Strategy:
  - Shard the batch dim (B=65536) across 8 cores (8192 each), replicate weights.
  - All gate matmuls fused into one (1024 x 832) @ (832 x B) matmul
    (rows [i|f|c|o] gates, cols [x|h|c|topic], -w_ho folded, (c-gate,c)=0).
  - Matmuls run in fp8 e4m3 with MatmulPerfMode.DoubleRow: each instruction
    contracts TWO 128-row k-blocks at 0.5 cycles per output column — 4x the
    fp16 rate in the TRN2 cost model.
  - fp8 alone is too lossy (rel err 2.5e-2 > 2e-2 gate), so hi/lo error
    compensation, all terms carrying a global x16 weight scale (e4m3's lo
    parts underflow its 2^-9 subnormal floor without it; the x16 is undone
    for free by the ScalarE activation's input scale):
       A_k = fp8(16 W_k)            @ fp8(z_k)          (hi x hi)
       C_k = fp8(16 W_k - WH16_k)   @ fp8(z_k)          (w-lo correction)
       B_k = fp8(W_k)               @ fp8(16(z - zh)_k) (act-lo correction)
    A+C on all k-blocks; B on x and h; topic gets (A_t, B_t) in one
    DoubleRow pair. Measured rel err ~8e-3 (vs 1.9e-4 fp16 baseline).
  - c is shipped fp16 for the exact elementwise cc = cf*c + ...; outputs are
    written fp16 and upcast on the host (tolerance-irrelevant, halves
    output DMA traffic).
"""

import os
import numpy as np
import ml_dtypes

import concourse.bass as bass
import concourse.bacc as bacc
import concourse.mybir as mybir
from concourse.tile import TileContext
from concourse.bass_utils import run_bass_kernel_spmd

I, H, T, B = 256, 256, 64, 65536
NCORES = 8
BS = B // NCORES          # 8192 batch columns per core
NT = 512                  # columns per outer chunk (one full PSUM bank of fp32)
NH = 256                  # columns per DoubleRow pass (rhs free cap is 2*NH=512)
NCHUNK = BS // NT         # 16
MB = 8                    # m-blocks: 4 gates x 2

FP8 = mybir.dt.float8e4
FP16 = mybir.dt.float16
FP32 = mybir.dt.float32
SIG = mybir.ActivationFunctionType.Sigmoid
TANH = mybir.ActivationFunctionType.Tanh
DR = mybir.MatmulPerfMode.DoubleRow
E4NP = ml_dtypes.float8_e4m3

WSCALE = 16.0

# Weight-pair stack, in emission order. Entries: (kind, input) where kind
# selects WH16/WL16/WRAW and input selects the z pair tile.
#   A = WH16 @ zh, C = WL16 @ zh, B = WRAW @ zl16
PAIRS = [
    ("A", "x"), ("A", "h"), ("A", "c"),
    ("C", "x"), ("C", "h"), ("C", "c"),
    ("B", "x"), ("B", "h"),
]
NPAIR = len(PAIRS)
# pairs skipped for the candidate-gate m-blocks (no c input term)
SKIP_C = {i for i, (_, src) in enumerate(PAIRS) if src == "c"}

_PROGRAM = None
_LAST_RESULTS = None  # for test harness introspection


def _build_program():
    nc = bacc.Bacc()

    # [x; h; c; topic] hi fp8 rows: 832
    zhi = nc.declare_dram_parameter("zhi", [832, BS], FP8, isOutput=False)
    # [x; h; topic] lo16 fp8 rows: 576
    zlo = nc.declare_dram_parameter("zlo", [576, BS], FP8, isOutput=False)
    c16 = nc.declare_dram_parameter("c16", [2 * 128, BS], FP16, isOutput=False)
    wt = nc.declare_dram_parameter("wt", [128, NPAIR * 2048], FP8, isOutput=False)
    wtt = nc.declare_dram_parameter("wtt", [64, 2048], FP8, isOutput=False)
    biases = nc.declare_dram_parameter("biases", [128, MB], FP32, isOutput=False)
    ch_out = nc.declare_dram_parameter("ch", [H, BS], FP16, isOutput=True)
    cc_out = nc.declare_dram_parameter("cc", [H, BS], FP16, isOutput=True)

    with TileContext(nc) as tc:
        with (
            tc.tile_pool(name="const", bufs=1) as constp,
            tc.tile_pool(name="zin", bufs=3) as zp,
            tc.tile_pool(name="gates", bufs=2) as gp,
            tc.tile_pool(name="psum", bufs=1, space="PSUM") as pp,
        ):
            wsb = [constp.tile([128, 2, 1024], FP8, tag=f"w{p}", name=f"wsb{p}")
                   for p in range(NPAIR)]
            wst = constp.tile([64, 2, 1024], FP8, tag="wt", name="wst")
            bsb = constp.tile([128, MB], FP32, tag="b", name="bsb")
            # first pair's weights immediately; the rest interleaved between
            # the first chunk's input DMAs (8 HWDGE queues run in parallel)
            nc.sync.dma_start(out=wsb[0][:], in_=wt[:, 0:2048])
            pending_w = list(range(1, NPAIR)) + [None]

            def weight_dma():
                if not pending_w:
                    return
                p = pending_w.pop(0)
                if p is None:
                    nc.sync.dma_start(out=wst[:], in_=wtt[:])
                    nc.sync.dma_start(out=bsb[:], in_=biases[:])
                else:
                    nc.sync.dma_start(out=wsb[p][:],
                                      in_=wt[:, p * 2048:(p + 1) * 2048])

            # PE warm-up: ~2.7us of tiny matmuls hidden under the initial DMA
            # fill releases the p-state ramp (cold PE runs slow for its first
            # ~3us of activity) before the real stream begins.
            wz = constp.tile([128, 64], FP16, tag="wz", name="wz")
            nc.vector.memset(wz[:], 0.0)
            # warm-up shares the ps0 bank; pool deps serialize it before chunk 0
            pdum = pp.tile([128, NT], FP32, tag="ps0", name="pdum")
            for _ in range(28):
                nc.tensor.matmul(pdum[0:64, 0:64], wz[:, 0:64], wz[:, 0:64],
                                 start=True, stop=True)

            for n in range(NCHUNK):
                c0 = n * NT
                c1 = c0 + NT

                # ---- load z pair tiles for this column chunk ----
                def pair_tile(nm, src, rows, npart=128):
                    t = zp.tile([npart, 2, NT], FP8, tag=nm, name=f"{nm}_{n}")
                    nc.sync.dma_start(out=t[:, 0, :], in_=src[rows[0]:rows[0] + npart, c0:c1])
                    if n == 0:
                        weight_dma()
                    nc.sync.dma_start(out=t[:, 1, :], in_=src[rows[1]:rows[1] + npart, c0:c1])
                    if n == 0:
                        weight_dma()
                    return t

                zx = pair_tile("zx", zhi, (0, 128))
                zh = pair_tile("zh", zhi, (256, 384))
                zc = pair_tile("zc", zhi, (512, 640))
                zlx = pair_tile("zlx", zlo, (0, 128))
                zlh = pair_tile("zlh", zlo, (256, 384))
                # topic pair: hi slot from zhi rows 768:832, lo slot from zlo 512:576
                zt = zp.tile([64, 2, NT], FP8, tag="zt", name=f"zt_{n}")
                nc.sync.dma_start(out=zt[:, 0, :], in_=zhi[768:832, c0:c1])
                nc.sync.dma_start(out=zt[:, 1, :], in_=zlo[512:576, c0:c1])
                ct = zp.tile([128, 2, NT], FP16, tag="ct", name=f"ct_{n}")
                nc.sync.dma_start(out=ct[:, 0, :], in_=c16[0:128, c0:c1])
                nc.sync.dma_start(out=ct[:, 1, :], in_=c16[128:256, c0:c1])
                while n == 0 and pending_w:
                    weight_dma()

                ztile = {"x": zx, "h": zh, "c": zc, "lx": zlx, "lh": zlh}

                # ---- the fused gate matmul: DoubleRow pairs ----
                # Each m-block owns one full PSUM bank [128, 512], filled by
                # two 256-col DoubleRow passes in a single accumulation group
                # (the first start=True pending-zeroes the whole bank).
                ps = []
                for m in range(MB):
                    pst = pp.tile([128, NT], FP32, tag=f"ps{m}", name=f"ps{m}_{n}")
                    ms, me = m * 128, (m + 1) * 128
                    plist = [p for p in range(NPAIR)
                             if not (m in (4, 5) and p in SKIP_C)]
                    for colh in range(2):
                        s = slice(colh * NH, (colh + 1) * NH)
                        for i, p in enumerate(plist):
                            kind, src = PAIRS[p]
                            z = ztile[src if kind != "B" else "l" + src]
                            nc.tensor.matmul(pst[:, s], wsb[p][:, :, ms:me],
                                             z[:, :, s],
                                             start=(colh == 0 and i == 0),
                                             stop=False, perf_mode=DR)
                        nc.tensor.matmul(pst[:, s], wst[:, :, ms:me], zt[:, :, s],
                                         start=False, stop=(colh == 1),
                                         perf_mode=DR)
                    ps.append(pst)

                # ---- gate activations (x16 weight scale undone here) ----
                def act(m, fn, nm):
                    t = gp.tile([128, NT], FP32, tag=nm, name=f"{nm}_{n}")
                    nc.scalar.activation(out=t[:], in_=ps[m][:], func=fn,
                                         bias=bsb[:, m:m + 1],
                                         scale=1.0 / WSCALE)
                    return t

                for half in range(2):
                    ci = act(0 + half, SIG, f"ci{half}")
                    cf = act(2 + half, SIG, f"cf{half}")
                    tg = act(4 + half, TANH, f"tg{half}")
                    co = act(6 + half, SIG, f"co{half}")

                    t1 = gp.tile([128, NT], FP32, tag=f"t1{half}", name=f"t1{half}_{n}")
                    nc.vector.tensor_mul(t1[:], ci[:], tg[:])
                    t2 = gp.tile([128, NT], FP32, tag=f"t2{half}", name=f"t2{half}_{n}")
                    nc.vector.tensor_mul(t2[:], cf[:], ct[:, half, :])
                    cct = gp.tile([128, NT], FP16, tag=f"cc{half}", name=f"cc{half}_{n}")
                    nc.vector.tensor_add(cct[:], t1[:], t2[:])
                    tcc = gp.tile([128, NT], FP32, tag=f"tcc{half}", name=f"tcc{half}_{n}")
                    nc.scalar.activation(out=tcc[:], in_=cct[:], func=TANH)
                    cht = gp.tile([128, NT], FP16, tag=f"chh{half}", name=f"chh{half}_{n}")
                    nc.vector.tensor_mul(cht[:], co[:], tcc[:])
                    r0, r1 = half * 128, (half + 1) * 128
                    nc.sync.dma_start(out=cc_out[r0:r1, c0:c1], in_=cct[:])
                    nc.sync.dma_start(out=ch_out[r0:r1, c0:c1], in_=cht[:])

    nc.finalize()
    return nc


def _q8(a):
    return a.astype(E4NP)


def _prep_weights(inp):
    """Fused (1024, 832) weights -> hi16/lo16/raw fp8 pair stacks."""
    Wf = np.zeros((1024, 832), np.float32)

    def put(g, blocks):
        r = g * 256
        for j, wb in enumerate(blocks):
            if wb is None:
                continue
            col = j * 256
            Wf[r:r + 256, col:col + wb.shape[1]] = wb

    put(0, [inp["w_ii"], inp["w_hi"], inp["w_ci"], inp["w_bi"]])
    put(1, [inp["w_if"], inp["w_hf"], inp["w_cf"], inp["w_bf"]])
    put(2, [inp["w_ic"], inp["w_hc"], None, inp["w_bc"]])
    put(3, [inp["w_io"], -inp["w_ho"], inp["w_co"], inp["w_bo"]])

    wT = Wf.T  # [832, 1024] k-major
    wh16 = _q8(WSCALE * wT)
    wl16 = _q8(WSCALE * wT - wh16.astype(np.float32))
    wraw = _q8(wT)

    kblk = {"x": (0, 128), "h": (256, 384), "c": (512, 640)}
    stacks = {"A": wh16, "C": wl16, "B": wraw}
    wt_host = np.zeros((128, NPAIR, 2, 1024), E4NP)
    for p, (kind, src) in enumerate(PAIRS):
        r0, r1 = kblk[src]
        wt_host[:, p, 0, :] = stacks[kind][r0:r0 + 128]
        wt_host[:, p, 1, :] = stacks[kind][r1:r1 + 128]
    wt_host = np.ascontiguousarray(wt_host.reshape(128, NPAIR * 2048))

    wtt_host = np.zeros((64, 2, 1024), E4NP)
    wtt_host[:, 0, :] = wh16[768:832]   # A_t
    wtt_host[:, 1, :] = wraw[768:832]   # B_t
    wtt_host = np.ascontiguousarray(wtt_host.reshape(64, 2048))

    bias_all = np.concatenate(
        [inp["bias_i"], inp["bias_f"], inp["bias_c"], inp["bias_o"]], axis=0
    ).reshape(MB, 128)
    bias_host = np.ascontiguousarray(bias_all.T).astype(np.float32)
    return wt_host, wtt_host, bias_host


def kernel(**inputs):
    global _PROGRAM, _LAST_RESULTS
    if _PROGRAM is None:
        _PROGRAM = _build_program()
    nc = _PROGRAM

    inp = {k: np.asarray(v, dtype=np.float32) for k, v in inputs.items()}
    wt_host, wtt_host, bias_host = _prep_weights(inp)

    zfull = np.concatenate(
        [inp["x"], inp["h"], inp["c"], inp["topic"]], axis=0)  # [832, B]
    zhi_all = _q8(zfull)
    res16 = WSCALE * (zfull - zhi_all.astype(np.float32))
    zlo_all = _q8(np.concatenate([res16[0:512], res16[768:832]], axis=0))
    c16_all = inp["c"].astype(np.float16)

    in_maps = []
    for i in range(NCORES):
        sl = slice(i * BS, (i + 1) * BS)
        in_maps.append({
            "zhi": np.ascontiguousarray(zhi_all[:, sl]),
            "zlo": np.ascontiguousarray(zlo_all[:, sl]),
            "c16": np.ascontiguousarray(c16_all[:, sl]),
            "wt": wt_host,
            "wtt": wtt_host,
            "biases": bias_host,
        })

    res = run_bass_kernel_spmd(
        nc, in_maps, list(range(NCORES)),
        trace=bool(os.environ.get("KERNEL_TRACE")),
    )
    _LAST_RESULTS = res

    ch = np.concatenate(
        [res.results[i]["ch"].astype(np.float32) for i in range(NCORES)], axis=1)
    cc = np.concatenate(
        [res.results[i]["cc"].astype(np.float32) for i in range(NCORES)], axis=1)
    return np.stack([ch, cc], axis=0)


# revision 5
# speedup vs baseline: 1.4161x; 1.2111x over previous
"""Contextual LSTM cell on 8 Trainium2 NeuronCores — fp8 DoubleRow edition.

Strategy:
  - Shard the batch dim (B=65536) across 8 cores (8192 each), replicate weights.
  - All gate matmuls fused into one (1024 x 832) @ (832 x B) matmul
    (rows [i|f|c|o] gates, cols [x|h|c|topic], -w_ho folded, (c-gate,c)=0).
  - Matmuls run in fp8 e4m3 with MatmulPerfMode.DoubleRow: each instruction
    contracts TWO 128-row k-blocks at 0.5 cycles per output column — 4x the
    fp16 rate in the TRN2 cost model.
  - fp8 alone is too lossy (rel err 2.5e-2 > 2e-2 gate), so hi/lo error
    compensation, all terms carrying a global x16 weight scale (e4m3's lo
    parts underflow its 2^-9 subnormal floor without it; the x16 is undone
    for free by the ScalarE activation's input scale):
       A_k = fp8(16 W_k)            @ fp8(z_k)          (hi x hi)
       C_k = fp8(16 W_k - WH16_k)   @ fp8(z_k)          (w-lo correction)
       B_k = fp8(W_k)               @ fp8(16(z - zh)_k) (act-lo correction)
    A+C on all k-blocks; B on x and h; topic gets (A_t, B_t) in one
    DoubleRow pair. Measured rel err ~8e-3 (vs 1.9e-4 fp16 baseline).
  - Biases ride a constant-one 65th partition row of the topic pair, so the
    gate activations need no per-half bias reads and process both gate
    halves (2 PSUM banks) in one wide instruction.
  - Everything downstream of PSUM is fp16 (packed SBUF fp16 runs the DVE in
    its 4x perf mode); c is shipped fp16 for the elementwise cc = cf*c.
  - All per-chunk DMA is consolidated: host pre-tiles inputs into
    per-chunk-contiguous planes (one ~2us fixed cost per DMA instruction in
    the TRN2 model makes many small DMAs expensive), outputs land in a
    chunk-tiled fp16 layout un-tiled on the host.
"""

import os
import numpy as np
import ml_dtypes

import concourse.bass as bass
import concourse.bacc as bacc
import concourse.mybir as mybir
from concourse.tile import TileContext
from concourse.bass_utils import run_bass_kernel_spmd

I, H, T, B = 256, 256, 64, 65536
NCORES = 8
BS = B // NCORES          # 8192 batch columns per core
NT = 512                  # columns per outer chunk (one full PSUM bank of fp32)
NH = 256                  # columns per DoubleRow pass (rhs free cap is 2*NH=512)
NCHUNK = BS // NT         # 16

FP8 = mybir.dt.float8e4
FP16 = mybir.dt.float16
FP32 = mybir.dt.float32
SIG = mybir.ActivationFunctionType.Sigmoid
TANH = mybir.ActivationFunctionType.Tanh
DR = mybir.MatmulPerfMode.DoubleRow
E4NP = ml_dtypes.float8_e4m3

WSCALE = 16.0

# Weight-pair stack, in emission order. Entries: (kind, input) where kind
# selects WH16/WL16/WRAW and input selects the z pair slots in the fused
# per-chunk z tile: x=(0,1) h=(2,3) c=(4,5) lx=(6,7) lh=(8,9).
#   A = WH16 @ zh, C = WL16 @ zh, B = WRAW @ zl16
PAIRS = [
    ("A", "x"), ("A", "h"), ("A", "c"),
    ("C", "x"), ("C", "h"), ("C", "c"),
    ("B", "x"), ("B", "h"),
]
NPAIR = len(PAIRS)
ZSLOT = {"x": 0, "h": 2, "c": 4, "lx": 6, "lh": 8}
NZSLOT = 10
# pairs skipped for the candidate-gate m-blocks (no c input term)
SKIP_C = {i for i, (_, src) in enumerate(PAIRS) if src == "c"}
GATE_FN = [SIG, SIG, TANH, SIG]   # i, f, c~, o

_PROGRAM = None
_LAST_RESULTS = None  # for test harness introspection


def _build_program():
    nc = bacc.Bacc()

    # fused fp8 z plane: per chunk 10 slots x 512 cols per partition
    zall = nc.declare_dram_parameter("zall", [128, NCHUNK * NZSLOT * NT], FP8,
                                     isOutput=False)
    # topic pair plane (+ constant-one bias row at partition 64)
    ztp = nc.declare_dram_parameter("ztp", [65, NCHUNK * 2 * NT], FP8,
                                    isOutput=False)
    c16 = nc.declare_dram_parameter("c16", [128, NCHUNK * 2 * NT], FP16,
                                    isOutput=False)
    wt = nc.declare_dram_parameter("wt", [128, NPAIR * 2048], FP8, isOutput=False)
    wtt = nc.declare_dram_parameter("wtt", [65, 2048], FP8, isOutput=False)
    ch_out = nc.declare_dram_parameter("ch", [128, NCHUNK * 2 * NT], FP16,
                                       isOutput=True)
    cc_out = nc.declare_dram_parameter("cc", [128, NCHUNK * 2 * NT], FP16,
                                       isOutput=True)

    with TileContext(nc) as tc:
        with (
            tc.tile_pool(name="const", bufs=1) as constp,
            tc.tile_pool(name="zin", bufs=3) as zp,
            tc.tile_pool(name="gates", bufs=2) as gp,
            tc.tile_pool(name="psum", bufs=1, space="PSUM") as pp,
        ):
            wsb = [constp.tile([128, 2, 1024], FP8, tag=f"w{p}", name=f"wsb{p}")
                   for p in range(NPAIR)]
            wst = constp.tile([65, 2, 1024], FP8, tag="wt", name="wst")
            # first pair's weights immediately; the rest interleaved between
            # the first chunks' input DMAs (HWDGE queues run in parallel)
            nc.sync.dma_start(out=wsb[0][:], in_=wt[:, 0:2048])
            pending_w = list(range(1, NPAIR)) + [None]

            def weight_dma():
                if not pending_w:
                    return
                p = pending_w.pop(0)
                if p is None:
                    nc.sync.dma_start(out=wst[:], in_=wtt[:])
                else:
                    nc.sync.dma_start(out=wsb[p][:],
                                      in_=wt[:, p * 2048:(p + 1) * 2048])

            # PE warm-up: ~2.7us of tiny matmuls hidden under the initial DMA
            # fill releases the p-state ramp (cold PE runs slow for its first
            # ~3us of activity) before the real stream begins.
            wz = constp.tile([128, 64], FP16, tag="wz", name="wz")
            nc.vector.memset(wz[:], 0.0)
            # warm-up shares the gate-0 PSUM banks; pool deps order it first
            pdum = pp.tile([128, 2, NT], FP32, tag="pg0", name="pdum")
            for _ in range(28):
                nc.tensor.matmul(pdum[0:64, 0, 0:64], wz[:, 0:64], wz[:, 0:64],
                                 start=True, stop=True)

            for n in range(NCHUNK):
                # ---- one consolidated DMA per input plane per chunk ----
                zt = zp.tile([128, NZSLOT, NT], FP8, tag="z", name=f"z_{n}")
                nc.sync.dma_start(
                    out=zt[:], in_=zall[:, n * NZSLOT * NT:(n + 1) * NZSLOT * NT])
                tt = zp.tile([65, 2, NT], FP8, tag="t", name=f"t_{n}")
                nc.sync.dma_start(
                    out=tt[:], in_=ztp[:, n * 2 * NT:(n + 1) * 2 * NT])
                ct = zp.tile([128, 2, NT], FP16, tag="c", name=f"c_{n}")
                nc.sync.dma_start(
                    out=ct[:], in_=c16[:, n * 2 * NT:(n + 1) * 2 * NT])
                while n == 0 and pending_w:
                    weight_dma()

                # ---- fused gate matmul: DoubleRow pairs ----
                # Gate g owns a [128, 2, 512] PSUM tile = one bank per gate
                # half; each (half, colh) quarter is one DoubleRow pass.
                pg = []
                for g in range(4):
                    pst = pp.tile([128, 2, NT], FP32, tag=f"pg{g}", name=f"pg{g}_{n}")
                    for hf in range(2):
                        m = 2 * g + hf
                        ms, me = m * 128, (m + 1) * 128
                        plist = [p for p in range(NPAIR)
                                 if not (g == 2 and p in SKIP_C)]
                        for colh in range(2):
                            cs = slice(colh * NH, (colh + 1) * NH)
                            for i, p in enumerate(plist):
                                kind, src = PAIRS[p]
                                zs = ZSLOT[src if kind != "B" else "l" + src]
                                nc.tensor.matmul(
                                    pst[:, hf, cs], wsb[p][:, :, ms:me],
                                    zt[:, zs:zs + 2, cs],
                                    start=(colh == 0 and i == 0),
                                    stop=False, perf_mode=DR)
                            # topic pair also carries the bias row
                            nc.tensor.matmul(
                                pst[:, hf, cs], wst[:, :, ms:me],
                                tt[:, :, cs],
                                start=False, stop=(colh == 1), perf_mode=DR)
                    pg.append(pst)

                # ---- wide gate activations (x16 weight scale undone here) ----
                def act(g, nm, in_=None):
                    t = gp.tile([128, 2, NT], FP16, tag=nm, name=f"{nm}_{n}")
                    nc.scalar.activation(out=t[:], in_=in_ if in_ is not None
                                         else pg[g][:], func=GATE_FN[g] if in_ is None else TANH,
                                         scale=1.0 / WSCALE if in_ is None else 1.0)
                    return t

                ci = act(0, "ci")
                cf = act(1, "cf")
                tg = act(2, "tg")
                co = act(3, "co")

                # ---- fp16 elementwise (DVE 4x perf mode) ----
                t1 = gp.tile([128, 2, NT], FP16, tag="t1", name=f"t1_{n}")
                nc.vector.tensor_mul(t1[:], ci[:], tg[:])
                t2 = gp.tile([128, 2, NT], FP16, tag="t2", name=f"t2_{n}")
                nc.vector.tensor_mul(t2[:], cf[:], ct[:])
                cct = gp.tile([128, 2, NT], FP16, tag="cc", name=f"cc_{n}")
                nc.vector.tensor_add(cct[:], t1[:], t2[:])
                tcc = act(None, "tcc", in_=cct)
                cht = gp.tile([128, 2, NT], FP16, tag="chh", name=f"chh_{n}")
                nc.vector.tensor_mul(cht[:], co[:], tcc[:])
                nc.sync.dma_start(
                    out=cc_out[:, n * 2 * NT:(n + 1) * 2 * NT], in_=cct[:])
                nc.sync.dma_start(
                    out=ch_out[:, n * 2 * NT:(n + 1) * 2 * NT], in_=cht[:])

    nc.finalize()
    return nc


def _q8(a):
    return a.astype(E4NP)


def _prep_weights(inp):
    """Fused (1024, 832) weights -> hi16/lo16/raw fp8 pair stacks."""
    Wf = np.zeros((1024, 832), np.float32)

    def put(g, blocks):
        r = g * 256
        for j, wb in enumerate(blocks):
            if wb is None:
                continue
            col = j * 256
            Wf[r:r + 256, col:col + wb.shape[1]] = wb

    put(0, [inp["w_ii"], inp["w_hi"], inp["w_ci"], inp["w_bi"]])
    put(1, [inp["w_if"], inp["w_hf"], inp["w_cf"], inp["w_bf"]])
    put(2, [inp["w_ic"], inp["w_hc"], None, inp["w_bc"]])
    put(3, [inp["w_io"], -inp["w_ho"], inp["w_co"], inp["w_bo"]])

    wT = Wf.T  # [832, 1024] k-major
    wh16 = _q8(WSCALE * wT)
    wl16 = _q8(WSCALE * wT - wh16.astype(np.float32))
    wraw = _q8(wT)

    kblk = {"x": (0, 128), "h": (256, 384), "c": (512, 640)}
    stacks = {"A": wh16, "C": wl16, "B": wraw}
    wt_host = np.zeros((128, NPAIR, 2, 1024), E4NP)
    for p, (kind, src) in enumerate(PAIRS):
        r0, r1 = kblk[src]
        wt_host[:, p, 0, :] = stacks[kind][r0:r0 + 128]
        wt_host[:, p, 1, :] = stacks[kind][r1:r1 + 128]
    wt_host = np.ascontiguousarray(wt_host.reshape(128, NPAIR * 2048))

    bias_vec = np.concatenate(
        [inp["bias_i"], inp["bias_f"], inp["bias_c"], inp["bias_o"]],
        axis=0).reshape(1024)
    wtt_host = np.zeros((65, 2, 1024), E4NP)
    wtt_host[:64, 0, :] = wh16[768:832]           # A_t
    wtt_host[:64, 1, :] = wraw[768:832]           # B_t
    wtt_host[64, 0, :] = _q8(WSCALE * bias_vec)   # bias rides slot 0
    wtt_host = np.ascontiguousarray(wtt_host.reshape(65, 2048))
    return wt_host, wtt_host


def _chunk_tile(a):
    """[R, BS] -> [R, NCHUNK, BS//NCHUNK] view of per-chunk columns."""
    return a.reshape(a.shape[0], NCHUNK, NT)


def kernel(**inputs):
    global _PROGRAM, _LAST_RESULTS
    if _PROGRAM is None:
        _PROGRAM = _build_program()
    nc = _PROGRAM

    inp = {k: np.asarray(v, dtype=np.float32) for k, v in inputs.items()}
    wt_host, wtt_host = _prep_weights(inp)

    zfull = np.concatenate(
        [inp["x"], inp["h"], inp["c"], inp["topic"]], axis=0)  # [832, B]
    zhi_all = _q8(zfull)
    res16 = _q8(WSCALE * (zfull - zhi_all.astype(np.float32)))
    c16_all = inp["c"].astype(np.float16)

    in_maps = []
    for i in range(NCORES):
        sl = slice(i * BS, (i + 1) * BS)
        zhi = zhi_all[:, sl]
        zlo = res16[:, sl]
        # z slots: x0 x1 h0 h1 c0 c1 | lx0 lx1 lh0 lh1
        slots = [zhi[r:r + 128] for r in range(0, 768, 128)] + \
                [zlo[r:r + 128] for r in range(0, 512, 128)]
        za = np.stack([_chunk_tile(s) for s in slots], axis=2)  # [128,NCHUNK,10,512]
        za = np.ascontiguousarray(za.reshape(128, NCHUNK * NZSLOT * NT))

        tp = np.empty((65, NCHUNK, 2, NT), E4NP)
        tp[:64, :, 0, :] = _chunk_tile(zhi[768:832])
        tp[:64, :, 1, :] = _chunk_tile(zlo[768:832])
        tp[64, :, 0, :] = np.float32(1.0)
        tp[64, :, 1, :] = np.float32(0.0)
        tp = np.ascontiguousarray(tp.reshape(65, NCHUNK * 2 * NT))

        cfull = c16_all[:, sl]
        cm = np.stack([_chunk_tile(cfull[0:128]), _chunk_tile(cfull[128:256])],
                      axis=2)  # [128, NCHUNK, 2, 512]
        cm = np.ascontiguousarray(cm.reshape(128, NCHUNK * 2 * NT))

        in_maps.append({
            "zall": za, "ztp": tp, "c16": cm,
            "wt": wt_host, "wtt": wtt_host,
        })

    res = run_bass_kernel_spmd(
        nc, in_maps, list(range(NCORES)),
        trace=bool(os.environ.get("KERNEL_TRACE")),
    )
    _LAST_RESULTS = res

    def untile(name):
        parts = []
        for i in range(NCORES):
            a = res.results[i][name].astype(np.float32)
            a = a.reshape(128, NCHUNK, 2, NT).transpose(2, 0, 1, 3)
            parts.append(a.reshape(256, BS))
        return np.concatenate(parts, axis=1)

    return np.stack([untile("ch"), untile("cc")], axis=0)


# revision 6
# speedup vs baseline: 1.4789x; 1.0444x over previous
"""Contextual LSTM cell on 8 Trainium2 NeuronCores — fp8 DoubleRow edition.

Strategy:
  - Shard the batch dim (B=65536) across 8 cores (8192 each), replicate weights.
  - All gate matmuls fused into one (1024 x 832) @ (832 x B) matmul
    (rows [i|f|c|o] gates, cols [x|h|c|topic], -w_ho folded, (c-gate,c)=0).
  - Matmuls run in fp8 e4m3 with MatmulPerfMode.DoubleRow: each instruction
    contracts TWO 128-row k-blocks at 0.5 cycles per output column — 4x the
    fp16 rate in the TRN2 cost model.
  - fp8 alone is too lossy (rel err 2.5e-2 > 2e-2 gate), so hi/lo error
    compensation, all terms carrying a global x16 weight scale (e4m3's lo
    parts underflow its 2^-9 subnormal floor without it; the x16 is undone
    for free by the ScalarE activation's input scale):
       A_k = fp8(16 W_k)            @ fp8(z_k)          (hi x hi)
       C_k = fp8(16 W_k - WH16_k)   @ fp8(z_k)          (w-lo correction)
       B_k = fp8(W_k)               @ fp8(16(z - zh)_k) (act-lo correction)
    A+C on all k-blocks; B on x and h; topic gets (A_t, B_t) in one
    DoubleRow pair. Measured rel err ~8e-3 (vs 1.9e-4 fp16 baseline).
  - Biases ride a constant-one 65th partition row of the topic pair, so the
    gate activations need no per-half bias reads and process both gate
    halves (2 PSUM banks) in one wide instruction.
  - Everything downstream of PSUM is fp16 (packed SBUF fp16 runs the DVE in
    its 4x perf mode); c is shipped fp16 for the elementwise cc = cf*c.
  - All per-chunk DMA is consolidated: host pre-tiles inputs into
    per-chunk-contiguous planes (one ~2us fixed cost per DMA instruction in
    the TRN2 model makes many small DMAs expensive), outputs land in a
    chunk-tiled fp16 layout un-tiled on the host.
"""

import os
import numpy as np
import ml_dtypes

import concourse.bass as bass
import concourse.bacc as bacc
import concourse.mybir as mybir
from concourse.tile import TileContext
from concourse.bass_utils import run_bass_kernel_spmd

I, H, T, B = 256, 256, 64, 65536
NCORES = 8
BS = B // NCORES          # 8192 batch columns per core
NT = 512                  # columns per outer chunk (one full PSUM bank of fp32)
NH = 256                  # columns per DoubleRow pass (rhs free cap is 2*NH=512)
NCHUNK = BS // NT         # 16

FP8 = mybir.dt.float8e4
FP16 = mybir.dt.float16
FP32 = mybir.dt.float32
SIG = mybir.ActivationFunctionType.Sigmoid
TANH = mybir.ActivationFunctionType.Tanh
DR = mybir.MatmulPerfMode.DoubleRow
E4NP = ml_dtypes.float8_e4m3

WSCALE = 16.0

# Weight-pair stack, in emission order. Entries: (kind, input) where kind
# selects WH16/WL16/WRAW and input selects the z pair slots in the fused
# per-chunk z tile: x=(0,1) h=(2,3) c=(4,5) lx=(6,7) lh=(8,9).
#   A = WH16 @ zh, C = WL16 @ zh, B = WRAW @ zl16
PAIRS = [
    ("A", "x"), ("A", "h"), ("A", "c"),
    ("C", "x"), ("C", "h"), ("C", "c"),
    ("B", "x"), ("B", "h"),
]
NPAIR = len(PAIRS)
ZSLOT = {"x": 0, "h": 2, "c": 4, "lx": 6, "lh": 8}
NZSLOT = 10
# pairs skipped for the candidate-gate m-blocks (no c input term)
SKIP_C = {i for i, (_, src) in enumerate(PAIRS) if src == "c"}
GATE_FN = [SIG, SIG, TANH, SIG]   # i, f, c~, o

_PROGRAM = None
_LAST_RESULTS = None  # for test harness introspection


def _build_program():
    nc = bacc.Bacc()

    # fused fp8 z plane: per chunk 10 slots x 512 cols per partition
    zall = nc.declare_dram_parameter("zall", [128, NCHUNK * NZSLOT * NT], FP8,
                                     isOutput=False)
    # topic pair plane (+ constant-one bias row at partition 64)
    ztp = nc.declare_dram_parameter("ztp", [65, NCHUNK * 2 * NT], FP8,
                                    isOutput=False)
    c16 = nc.declare_dram_parameter("c16", [128, NCHUNK * 2 * NT], FP16,
                                    isOutput=False)
    wt = nc.declare_dram_parameter("wt", [128, NPAIR * 2048], FP8, isOutput=False)
    wtt = nc.declare_dram_parameter("wtt", [65, 2048], FP8, isOutput=False)
    ch_out = nc.declare_dram_parameter("ch", [128, NCHUNK * 2 * NT], FP16,
                                       isOutput=True)
    cc_out = nc.declare_dram_parameter("cc", [128, NCHUNK * 2 * NT], FP16,
                                       isOutput=True)

    with TileContext(nc) as tc:
        with (
            tc.tile_pool(name="const", bufs=1) as constp,
            tc.tile_pool(name="zin", bufs=3) as zp,
            tc.tile_pool(name="gates", bufs=2) as gp,
            tc.tile_pool(name="psum", bufs=1, space="PSUM") as pp,
        ):
            wsb = [constp.tile([128, 2, 1024], FP8, tag=f"w{p}", name=f"wsb{p}")
                   for p in range(NPAIR)]
            wst = constp.tile([65, 2, 1024], FP8, tag="wt", name="wst")
            # first pair's weights immediately; the rest interleaved between
            # the first chunks' input DMAs (HWDGE queues run in parallel)
            nc.scalar.dma_start(out=wsb[0][:], in_=wt[:, 0:2048])
            nc.scalar.dma_start(out=wst[:], in_=wtt[:])
            for p in range(1, NPAIR):
                nc.scalar.dma_start(out=wsb[p][:],
                                    in_=wt[:, p * 2048:(p + 1) * 2048])

            # PE warm-up: ~2.7us of tiny matmuls hidden under the initial DMA
            # fill releases the p-state ramp (cold PE runs slow for its first
            # ~3us of activity) before the real stream begins.
            wz = constp.tile([128, 64], FP16, tag="wz", name="wz")
            nc.vector.memset(wz[:], 0.0)
            # warm-up shares the gate-0 PSUM banks; pool deps order it first
            pdum = pp.tile([128, 2, NT], FP32, tag="pg0", name="pdum")
            for _ in range(28):
                nc.tensor.matmul(pdum[0:64, 0, 0:64], wz[:, 0:64], wz[:, 0:64],
                                 start=True, stop=True)

            for n in range(NCHUNK):
                # ---- one consolidated DMA per input plane per chunk ----
                zt = zp.tile([128, NZSLOT, NT], FP8, tag="z", name=f"z_{n}")
                z0 = n * NZSLOT * NT
                if n == 0:
                    # split the first z plane across two queues so the PE can
                    # start ~1us earlier
                    cut = 5 * NT
                    nc.sync.dma_start(out=zt[:, 0:5, :], in_=zall[:, z0:z0 + cut])
                    nc.gpsimd.dma_start(out=zt[:, 5:NZSLOT, :],
                                        in_=zall[:, z0 + cut:z0 + NZSLOT * NT])
                else:
                    nc.sync.dma_start(out=zt[:], in_=zall[:, z0:z0 + NZSLOT * NT])
                tt = zp.tile([65, 2, NT], FP8, tag="t", name=f"t_{n}")
                nc.sync.dma_start(
                    out=tt[:], in_=ztp[:, n * 2 * NT:(n + 1) * 2 * NT])
                ct = zp.tile([128, 2, NT], FP16, tag="c", name=f"c_{n}")
                nc.gpsimd.dma_start(
                    out=ct[:], in_=c16[:, n * 2 * NT:(n + 1) * 2 * NT])

                # ---- fused gate matmul: DoubleRow pairs ----
                # Gate g owns a [128, 2, 512] PSUM tile = one bank per gate
                # half; each (half, colh) quarter is one DoubleRow pass.
                # Chunk 0 is emitted pair-major (all A_x, topic, A_h, ...) to
                # match weight/z DMA arrival; later chunks gate-major so each
                # gate finishes early for its activation.
                pg = [pp.tile([128, 2, NT], FP32, tag=f"pg{g}", name=f"pg{g}_{n}")
                      for g in range(4)]
                TOPIC = -1
                ops = []  # (pair or TOPIC, g, hf, colh)
                if n == 0:
                    order = [0, TOPIC, 1, 2, 3, 4, 5, 6, 7]
                    for p in order:
                        for g in range(4):
                            if g == 2 and p in SKIP_C:
                                continue
                            for hf in range(2):
                                for colh in range(2):
                                    ops.append((p, g, hf, colh))
                else:
                    for g in range(4):
                        plist = [p for p in range(NPAIR)
                                 if not (g == 2 and p in SKIP_C)]
                        for hf in range(2):
                            for colh in range(2):
                                for p in plist + [TOPIC]:
                                    ops.append((p, g, hf, colh))
                started = set()
                last_op = {}
                for k, (p, g, hf, colh) in enumerate(ops):
                    last_op[(g, hf)] = k
                for k, (p, g, hf, colh) in enumerate(ops):
                    m = 2 * g + hf
                    ms, me = m * 128, (m + 1) * 128
                    cs = slice(colh * NH, (colh + 1) * NH)
                    if p == TOPIC:
                        lhsT, rhs = wst[:, :, ms:me], tt[:, :, cs]
                    else:
                        kind, src = PAIRS[p]
                        zs = ZSLOT[src if kind != "B" else "l" + src]
                        lhsT, rhs = wsb[p][:, :, ms:me], zt[:, zs:zs + 2, cs]
                    key = (g, hf)
                    nc.tensor.matmul(pg[g][:, hf, cs], lhsT, rhs,
                                     start=(key not in started),
                                     stop=(last_op[key] == k), perf_mode=DR)
                    started.add(key)

                # ---- wide gate activations (x16 weight scale undone here) ----
                def act(g, nm, in_=None):
                    t = gp.tile([128, 2, NT], FP16, tag=nm, name=f"{nm}_{n}")
                    nc.scalar.activation(out=t[:], in_=in_ if in_ is not None
                                         else pg[g][:], func=GATE_FN[g] if in_ is None else TANH,
                                         scale=1.0 / WSCALE if in_ is None else 1.0)
                    return t

                ci = act(0, "ci")
                cf = act(1, "cf")
                tg = act(2, "tg")
                co = act(3, "co")

                # ---- fp16 elementwise (DVE 4x perf mode) ----
                t1 = gp.tile([128, 2, NT], FP16, tag="t1", name=f"t1_{n}")
                nc.vector.tensor_mul(t1[:], ci[:], tg[:])
                t2 = gp.tile([128, 2, NT], FP16, tag="t2", name=f"t2_{n}")
                nc.vector.tensor_mul(t2[:], cf[:], ct[:])
                cct = gp.tile([128, 2, NT], FP16, tag="cc", name=f"cc_{n}")
                nc.vector.tensor_add(cct[:], t1[:], t2[:])
                tcc = act(None, "tcc", in_=cct)
                cht = gp.tile([128, 2, NT], FP16, tag="chh", name=f"chh_{n}")
                nc.vector.tensor_mul(cht[:], co[:], tcc[:])
                nc.sync.dma_start(
                    out=cc_out[:, n * 2 * NT:(n + 1) * 2 * NT], in_=cct[:])
                nc.sync.dma_start(
                    out=ch_out[:, n * 2 * NT:(n + 1) * 2 * NT], in_=cht[:])

    nc.finalize()
    return nc


def _q8(a):
    return a.astype(E4NP)


def _prep_weights(inp):
    """Fused (1024, 832) weights -> hi16/lo16/raw fp8 pair stacks."""
    Wf = np.zeros((1024, 832), np.float32)

    def put(g, blocks):
        r = g * 256
        for j, wb in enumerate(blocks):
            if wb is None:
                continue
            col = j * 256
            Wf[r:r + 256, col:col + wb.shape[1]] = wb

    put(0, [inp["w_ii"], inp["w_hi"], inp["w_ci"], inp["w_bi"]])
    put(1, [inp["w_if"], inp["w_hf"], inp["w_cf"], inp["w_bf"]])
    put(2, [inp["w_ic"], inp["w_hc"], None, inp["w_bc"]])
    put(3, [inp["w_io"], -inp["w_ho"], inp["w_co"], inp["w_bo"]])

    wT = Wf.T  # [832, 1024] k-major
    wh16 = _q8(WSCALE * wT)
    wl16 = _q8(WSCALE * wT - wh16.astype(np.float32))
    wraw = _q8(wT)

    kblk = {"x": (0, 128), "h": (256, 384), "c": (512, 640)}
    stacks = {"A": wh16, "C": wl16, "B": wraw}
    wt_host = np.zeros((128, NPAIR, 2, 1024), E4NP)
    for p, (kind, src) in enumerate(PAIRS):
        r0, r1 = kblk[src]
        wt_host[:, p, 0, :] = stacks[kind][r0:r0 + 128]
        wt_host[:, p, 1, :] = stacks[kind][r1:r1 + 128]
    wt_host = np.ascontiguousarray(wt_host.reshape(128, NPAIR * 2048))

    bias_vec = np.concatenate(
        [inp["bias_i"], inp["bias_f"], inp["bias_c"], inp["bias_o"]],
        axis=0).reshape(1024)
    wtt_host = np.zeros((65, 2, 1024), E4NP)
    wtt_host[:64, 0, :] = wh16[768:832]           # A_t
    wtt_host[:64, 1, :] = wraw[768:832]           # B_t
    wtt_host[64, 0, :] = _q8(WSCALE * bias_vec)   # bias rides slot 0
    wtt_host = np.ascontiguousarray(wtt_host.reshape(65, 2048))
    return wt_host, wtt_host


def _chunk_tile(a):
    """[R, BS] -> [R, NCHUNK, BS//NCHUNK] view of per-chunk columns."""
    return a.reshape(a.shape[0], NCHUNK, NT)


def kernel(**inputs):
    global _PROGRAM, _LAST_RESULTS
    if _PROGRAM is None:
        _PROGRAM = _build_program()
    nc = _PROGRAM

    inp = {k: np.asarray(v, dtype=np.float32) for k, v in inputs.items()}
    wt_host, wtt_host = _prep_weights(inp)

    zfull = np.concatenate(
        [inp["x"], inp["h"], inp["c"], inp["topic"]], axis=0)  # [832, B]
    zhi_all = _q8(zfull)
    res16 = _q8(WSCALE * (zfull - zhi_all.astype(np.float32)))
    c16_all = inp["c"].astype(np.float16)

    in_maps = []
    for i in range(NCORES):
        sl = slice(i * BS, (i + 1) * BS)
        zhi = zhi_all[:, sl]
        zlo = res16[:, sl]
        # z slots: x0 x1 h0 h1 c0 c1 | lx0 lx1 lh0 lh1
        slots = [zhi[r:r + 128] for r in range(0, 768, 128)] + \
                [zlo[r:r + 128] for r in range(0, 512, 128)]
        za = np.stack([_chunk_tile(s) for s in slots], axis=2)  # [128,NCHUNK,10,512]
        za = np.ascontiguousarray(za.reshape(128, NCHUNK * NZSLOT * NT))

        tp = np.empty((65, NCHUNK, 2, NT), E4NP)
        tp[:64, :, 0, :] = _chunk_tile(zhi[768:832])
        tp[:64, :, 1, :] = _chunk_tile(zlo[768:832])
        tp[64, :, 0, :] = np.float32(1.0)
        tp[64, :, 1, :] = np.float32(0.0)
        tp = np.ascontiguousarray(tp.reshape(65, NCHUNK * 2 * NT))

        cfull = c16_all[:, sl]
        cm = np.stack([_chunk_tile(cfull[0:128]), _chunk_tile(cfull[128:256])],
                      axis=2)  # [128, NCHUNK, 2, 512]
        cm = np.ascontiguousarray(cm.reshape(128, NCHUNK * 2 * NT))

        in_maps.append({
            "zall": za, "ztp": tp, "c16": cm,
            "wt": wt_host, "wtt": wtt_host,
        })

    res = run_bass_kernel_spmd(
        nc, in_maps, list(range(NCORES)),
        trace=bool(os.environ.get("KERNEL_TRACE")),
    )
    _LAST_RESULTS = res

    def untile(name):
        parts = []
        for i in range(NCORES):
            a = res.results[i][name].astype(np.float32)
            a = a.reshape(128, NCHUNK, 2, NT).transpose(2, 0, 1, 3)
            parts.append(a.reshape(256, BS))
        return np.concatenate(parts, axis=1)

    return np.stack([untile("ch"), untile("cc")], axis=0)


# revision 7
# speedup vs baseline: 1.7552x; 1.1868x over previous
"""Contextual LSTM cell on 8 Trainium2 NeuronCores — fp8 DoubleRow edition.

Strategy:
  - Shard the batch dim (B=65536) across 8 cores (8192 each), replicate weights.
  - All gate matmuls fused into one (1024 x 832) @ (832 x B) matmul
    (rows [i|f|c|o] gates, cols [x|h|c|topic], -w_ho folded, (c-gate,c)=0).
  - Matmuls run in fp8 e4m3 with MatmulPerfMode.DoubleRow: each instruction
    contracts TWO 128-row k-blocks at 0.5 cycles per output column — 4x the
    fp16 rate in the TRN2 cost model.
  - fp8 alone is too lossy (rel err 2.5e-2 > 2e-2 gate), so hi/lo error
    compensation, all terms carrying a global x16 weight scale (e4m3's lo
    parts underflow its 2^-9 subnormal floor without it; the x16 is undone
    for free by the ScalarE activation's input scale):
       A_k = fp8(16 W_k)            @ fp8(z_k)          (hi x hi)
       C_k = fp8(16 W_k - WH16_k)   @ fp8(z_k)          (w-lo correction)
       B_k = fp8(W_k)               @ fp8(16(z - zh)_k) (act-lo correction)
    A+C on all k-blocks; B on x and h; topic gets (A_t, B_t) in one
    DoubleRow pair. Measured rel err ~8e-3 (vs 1.9e-4 fp16 baseline).
  - Biases ride a constant-one 65th partition row of the topic pair, so the
    gate activations need no per-half bias reads and process both gate
    halves (2 PSUM banks) in one wide instruction.
  - Everything downstream of PSUM is fp16 (packed SBUF fp16 runs the DVE in
    its 4x perf mode); c is shipped fp16 for the elementwise cc = cf*c.
  - All per-chunk DMA is consolidated: host pre-tiles inputs into
    per-chunk-contiguous planes (one ~2us fixed cost per DMA instruction in
    the TRN2 model makes many small DMAs expensive), outputs land in a
    chunk-tiled fp16 layout un-tiled on the host.
"""

import os
import numpy as np
import ml_dtypes

import concourse.bass as bass
import concourse.bacc as bacc
import concourse.mybir as mybir
from concourse.tile import TileContext
from concourse.bass_utils import run_bass_kernel_spmd

I, H, T, B = 256, 256, 64, 65536
NCORES = 8
BS = B // NCORES          # 8192 batch columns per core
NT = 512                  # columns per outer chunk (one full PSUM bank of fp32)
NH = 256                  # columns per DoubleRow pass (rhs free cap is 2*NH=512)
NCHUNK = BS // NT         # 16

FP8 = mybir.dt.float8e4
FP16 = mybir.dt.float16
FP32 = mybir.dt.float32
SIG = mybir.ActivationFunctionType.Sigmoid
TANH = mybir.ActivationFunctionType.Tanh
DR = mybir.MatmulPerfMode.DoubleRow
E4NP = ml_dtypes.float8_e4m3

WSCALE = 16.0

# Weight-pair stack, in emission order. Entries: (kind, input) where kind
# selects WH16/WL16/WRAW and input selects the z pair slots in the fused
# per-chunk z tile: x=(0,1) h=(2,3) c=(4,5) lx=(6,7) lh=(8,9).
#   A = WH16 @ zh, C = WL16 @ zh, B = WRAW @ zl16
PAIRS = [
    ("A", "x"), ("A", "h"), ("A", "c"),
    ("C", "x"), ("C", "h"), ("C", "c"),
    ("B", "x"), ("B", "h"),
]
NPAIR = len(PAIRS)
ZSLOT = {"x": 0, "h": 2, "c": 4, "lx": 6, "lh": 8}
NZSLOT = 10
# pairs skipped for the candidate-gate m-blocks (no c input term)
SKIP_C = {i for i, (_, src) in enumerate(PAIRS) if src == "c"}
# act-lo (B) compensation only pays off on the candidate gate: its tanh has
# unit derivative and feeds cc directly, so it dominates the act-quantization
# error; the sigmoid gates are damped enough to skip (measured ladder:
# B on all gates 8.2e-3, B on candidate only ~1.4e-2, no B 1.9e-2).
B_PAIRS = {i for i, (kind, _) in enumerate(PAIRS) if kind == "B"}


def gate_pairs(g):
    return [p for p in range(NPAIR)
            if not (g == 2 and p in SKIP_C) and not (g != 2 and p in B_PAIRS)]
GATE_FN = [SIG, SIG, TANH, SIG]   # i, f, c~, o

_PROGRAM = None
_LAST_RESULTS = None  # for test harness introspection


def _build_program():
    nc = bacc.Bacc()

    # fused fp8 z plane: per chunk 10 slots x 512 cols per partition
    zall = nc.declare_dram_parameter("zall", [128, NCHUNK * NZSLOT * NT], FP8,
                                     isOutput=False)
    # topic pair plane (+ constant-one bias row at partition 64)
    ztp = nc.declare_dram_parameter("ztp", [65, NCHUNK * 2 * NT], FP8,
                                    isOutput=False)
    c16 = nc.declare_dram_parameter("c16", [128, NCHUNK * 2 * NT], FP16,
                                    isOutput=False)
    wt = nc.declare_dram_parameter("wt", [128, NPAIR * 2048], FP8, isOutput=False)
    wtt = nc.declare_dram_parameter("wtt", [65, 2048], FP8, isOutput=False)
    ch_out = nc.declare_dram_parameter("ch", [128, NCHUNK * 2 * NT], FP16,
                                       isOutput=True)
    cc_out = nc.declare_dram_parameter("cc", [128, NCHUNK * 2 * NT], FP16,
                                       isOutput=True)

    with TileContext(nc) as tc:
        with (
            tc.tile_pool(name="const", bufs=1) as constp,
            tc.tile_pool(name="zin", bufs=3) as zp,
            tc.tile_pool(name="gates", bufs=2) as gp,
            tc.tile_pool(name="psum", bufs=1, space="PSUM") as pp,
        ):
            wsb = [constp.tile([128, 2, 1024], FP8, tag=f"w{p}", name=f"wsb{p}")
                   for p in range(NPAIR)]
            wst = constp.tile([65, 2, 1024], FP8, tag="wt", name="wst")
            # first pair's weights immediately; the rest interleaved between
            # the first chunks' input DMAs (HWDGE queues run in parallel)
            nc.scalar.dma_start(out=wsb[0][:], in_=wt[:, 0:2048])
            nc.scalar.dma_start(out=wst[:], in_=wtt[:])
            for p in range(1, NPAIR):
                nc.scalar.dma_start(out=wsb[p][:],
                                    in_=wt[:, p * 2048:(p + 1) * 2048])

            # PE warm-up: ~2.7us of tiny matmuls hidden under the initial DMA
            # fill releases the p-state ramp (cold PE runs slow for its first
            # ~3us of activity) before the real stream begins.
            wz = constp.tile([128, 64], FP16, tag="wz", name="wz")
            nc.vector.memset(wz[:], 0.0)
            # warm-up shares the gate-0 PSUM banks; pool deps order it first
            pdum = pp.tile([128, 2, NT], FP32, tag="pg0", name="pdum")
            for _ in range(28):
                nc.tensor.matmul(pdum[0:64, 0, 0:64], wz[:, 0:64], wz[:, 0:64],
                                 start=True, stop=True)

            for n in range(NCHUNK):
                # ---- one consolidated DMA per input plane per chunk ----
                zt = zp.tile([128, NZSLOT, NT], FP8, tag="z", name=f"z_{n}")
                z0 = n * NZSLOT * NT
                if n == 0:
                    # split the first z plane across two queues so the PE can
                    # start ~1us earlier
                    cut = 5 * NT
                    nc.sync.dma_start(out=zt[:, 0:5, :], in_=zall[:, z0:z0 + cut])
                    nc.gpsimd.dma_start(out=zt[:, 5:NZSLOT, :],
                                        in_=zall[:, z0 + cut:z0 + NZSLOT * NT])
                else:
                    nc.sync.dma_start(out=zt[:], in_=zall[:, z0:z0 + NZSLOT * NT])
                tt = zp.tile([65, 2, NT], FP8, tag="t", name=f"t_{n}")
                nc.sync.dma_start(
                    out=tt[:], in_=ztp[:, n * 2 * NT:(n + 1) * 2 * NT])
                ct = zp.tile([128, 2, NT], FP16, tag="c", name=f"c_{n}")
                nc.gpsimd.dma_start(
                    out=ct[:], in_=c16[:, n * 2 * NT:(n + 1) * 2 * NT])

                # ---- fused gate matmul: DoubleRow pairs ----
                # Gate g owns a [128, 2, 512] PSUM tile = one bank per gate
                # half; each (half, colh) quarter is one DoubleRow pass.
                # Chunk 0 is emitted pair-major (all A_x, topic, A_h, ...) to
                # match weight/z DMA arrival; later chunks gate-major so each
                # gate finishes early for its activation.
                pg = [pp.tile([128, 2, NT], FP32, tag=f"pg{g}", name=f"pg{g}_{n}")
                      for g in range(4)]
                TOPIC = -1
                ops = []  # (pair or TOPIC, g, hf, colh)
                if n == 0:
                    order = [0, TOPIC, 1, 2, 3, 4, 5, 6, 7]
                    for p in order:
                        for g in range(4):
                            if p != TOPIC and p not in gate_pairs(g):
                                continue
                            for hf in range(2):
                                for colh in range(2):
                                    ops.append((p, g, hf, colh))
                else:
                    for g in range(4):
                        for hf in range(2):
                            for colh in range(2):
                                for p in gate_pairs(g) + [TOPIC]:
                                    ops.append((p, g, hf, colh))
                started = set()
                last_op = {}
                for k, (p, g, hf, colh) in enumerate(ops):
                    last_op[(g, hf)] = k
                for k, (p, g, hf, colh) in enumerate(ops):
                    m = 2 * g + hf
                    ms, me = m * 128, (m + 1) * 128
                    cs = slice(colh * NH, (colh + 1) * NH)
                    if p == TOPIC:
                        lhsT, rhs = wst[:, :, ms:me], tt[:, :, cs]
                    else:
                        kind, src = PAIRS[p]
                        zs = ZSLOT[src if kind != "B" else "l" + src]
                        lhsT, rhs = wsb[p][:, :, ms:me], zt[:, zs:zs + 2, cs]
                    key = (g, hf)
                    nc.tensor.matmul(pg[g][:, hf, cs], lhsT, rhs,
                                     start=(key not in started),
                                     stop=(last_op[key] == k), perf_mode=DR)
                    started.add(key)

                # ---- wide gate activations (x16 weight scale undone here) ----
                def act(g, nm, in_=None):
                    t = gp.tile([128, 2, NT], FP16, tag=nm, name=f"{nm}_{n}")
                    nc.scalar.activation(out=t[:], in_=in_ if in_ is not None
                                         else pg[g][:], func=GATE_FN[g] if in_ is None else TANH,
                                         scale=1.0 / WSCALE if in_ is None else 1.0)
                    return t

                ci = act(0, "ci")
                cf = act(1, "cf")
                tg = act(2, "tg")
                co = act(3, "co")

                # ---- fp16 elementwise (DVE 4x perf mode) ----
                t1 = gp.tile([128, 2, NT], FP16, tag="t1", name=f"t1_{n}")
                nc.vector.tensor_mul(t1[:], ci[:], tg[:])
                t2 = gp.tile([128, 2, NT], FP16, tag="t2", name=f"t2_{n}")
                nc.vector.tensor_mul(t2[:], cf[:], ct[:])
                cct = gp.tile([128, 2, NT], FP16, tag="cc", name=f"cc_{n}")
                nc.vector.tensor_add(cct[:], t1[:], t2[:])
                tcc = act(None, "tcc", in_=cct)
                cht = gp.tile([128, 2, NT], FP16, tag="chh", name=f"chh_{n}")
                nc.vector.tensor_mul(cht[:], co[:], tcc[:])
                nc.sync.dma_start(
                    out=cc_out[:, n * 2 * NT:(n + 1) * 2 * NT], in_=cct[:])
                nc.sync.dma_start(
                    out=ch_out[:, n * 2 * NT:(n + 1) * 2 * NT], in_=cht[:])

    nc.finalize()
    return nc


def _q8(a):
    return a.astype(E4NP)


def _prep_weights(inp):
    """Fused (1024, 832) weights -> hi16/lo16/raw fp8 pair stacks."""
    Wf = np.zeros((1024, 832), np.float32)

    def put(g, blocks):
        r = g * 256
        for j, wb in enumerate(blocks):
            if wb is None:
                continue
            col = j * 256
            Wf[r:r + 256, col:col + wb.shape[1]] = wb

    put(0, [inp["w_ii"], inp["w_hi"], inp["w_ci"], inp["w_bi"]])
    put(1, [inp["w_if"], inp["w_hf"], inp["w_cf"], inp["w_bf"]])
    put(2, [inp["w_ic"], inp["w_hc"], None, inp["w_bc"]])
    put(3, [inp["w_io"], -inp["w_ho"], inp["w_co"], inp["w_bo"]])

    wT = Wf.T  # [832, 1024] k-major
    wh16 = _q8(WSCALE * wT)
    wl16 = _q8(WSCALE * wT - wh16.astype(np.float32))
    wraw = _q8(wT)

    kblk = {"x": (0, 128), "h": (256, 384), "c": (512, 640)}
    stacks = {"A": wh16, "C": wl16, "B": wraw}
    wt_host = np.zeros((128, NPAIR, 2, 1024), E4NP)
    for p, (kind, src) in enumerate(PAIRS):
        r0, r1 = kblk[src]
        wt_host[:, p, 0, :] = stacks[kind][r0:r0 + 128]
        wt_host[:, p, 1, :] = stacks[kind][r1:r1 + 128]
    wt_host = np.ascontiguousarray(wt_host.reshape(128, NPAIR * 2048))

    bias_vec = np.concatenate(
        [inp["bias_i"], inp["bias_f"], inp["bias_c"], inp["bias_o"]],
        axis=0).reshape(1024)
    wtt_host = np.zeros((65, 2, 1024), E4NP)
    wtt_host[:64, 0, :] = wh16[768:832]           # A_t
    wtt_host[:64, 1, :] = wraw[768:832]           # B_t
    wtt_host[64, 0, :] = _q8(WSCALE * bias_vec)   # bias rides slot 0
    wtt_host = np.ascontiguousarray(wtt_host.reshape(65, 2048))
    return wt_host, wtt_host


def _chunk_tile(a):
    """[R, BS] -> [R, NCHUNK, BS//NCHUNK] view of per-chunk columns."""
    return a.reshape(a.shape[0], NCHUNK, NT)


def kernel(**inputs):
    global _PROGRAM, _LAST_RESULTS
    if _PROGRAM is None:
        _PROGRAM = _build_program()
    nc = _PROGRAM

    inp = {k: np.asarray(v, dtype=np.float32) for k, v in inputs.items()}
    wt_host, wtt_host = _prep_weights(inp)

    zfull = np.concatenate(
        [inp["x"], inp["h"], inp["c"], inp["topic"]], axis=0)  # [832, B]
    zhi_all = _q8(zfull)
    res16 = _q8(WSCALE * (zfull - zhi_all.astype(np.float32)))
    c16_all = inp["c"].astype(np.float16)

    in_maps = []
    for i in range(NCORES):
        sl = slice(i * BS, (i + 1) * BS)
        zhi = zhi_all[:, sl]
        zlo = res16[:, sl]
        # z slots: x0 x1 h0 h1 c0 c1 | lx0 lx1 lh0 lh1
        slots = [zhi[r:r + 128] for r in range(0, 768, 128)] + \
                [zlo[r:r + 128] for r in range(0, 512, 128)]
        za = np.stack([_chunk_tile(s) for s in slots], axis=2)  # [128,NCHUNK,10,512]
        za = np.ascontiguousarray(za.reshape(128, NCHUNK * NZSLOT * NT))

        tp = np.empty((65, NCHUNK, 2, NT), E4NP)
        tp[:64, :, 0, :] = _chunk_tile(zhi[768:832])
        tp[:64, :, 1, :] = _chunk_tile(zlo[768:832])
        tp[64, :, 0, :] = np.float32(1.0)
        tp[64, :, 1, :] = np.float32(0.0)
        tp = np.ascontiguousarray(tp.reshape(65, NCHUNK * 2 * NT))

        cfull = c16_all[:, sl]
        cm = np.stack([_chunk_tile(cfull[0:128]), _chunk_tile(cfull[128:256])],
                      axis=2)  # [128, NCHUNK, 2, 512]
        cm = np.ascontiguousarray(cm.reshape(128, NCHUNK * 2 * NT))

        in_maps.append({
            "zall": za, "ztp": tp, "c16": cm,
            "wt": wt_host, "wtt": wtt_host,
        })

    res = run_bass_kernel_spmd(
        nc, in_maps, list(range(NCORES)),
        trace=bool(os.environ.get("KERNEL_TRACE")),
    )
    _LAST_RESULTS = res

    def untile(name):
        parts = []
        for i in range(NCORES):
            a = res.results[i][name].astype(np.float32)
            a = a.reshape(128, NCHUNK, 2, NT).transpose(2, 0, 1, 3)
            parts.append(a.reshape(256, BS))
        return np.concatenate(parts, axis=1)

    return np.stack([untile("ch"), untile("cc")], axis=0)


# revision 8
# speedup vs baseline: 1.7640x; 1.0050x over previous
"""Contextual LSTM cell on 8 Trainium2 NeuronCores — fp8 DoubleRow edition.

Strategy:
  - Shard the batch dim (B=65536) across 8 cores (8192 each), replicate weights.
  - All gate matmuls fused into one (1024 x 832) @ (832 x B) matmul
    (rows [i|f|c|o] gates, cols [x|h|c|topic], -w_ho folded, (c-gate,c)=0).
  - Matmuls run in fp8 e4m3 with MatmulPerfMode.DoubleRow: each instruction
    contracts TWO 128-row k-blocks at 0.5 cycles per output column — 4x the
    fp16 rate in the TRN2 cost model.
  - fp8 alone is too lossy (rel err 2.5e-2 > 2e-2 gate), so hi/lo error
    compensation, all terms carrying a global x16 weight scale (e4m3's lo
    parts underflow its 2^-9 subnormal floor without it; the x16 is undone
    for free by the ScalarE activation's input scale):
       A_k = fp8(16 W_k)            @ fp8(z_k)          (hi x hi)
       C_k = fp8(16 W_k - WH16_k)   @ fp8(z_k)          (w-lo correction)
       B_k = fp8(W_k)               @ fp8(16(z - zh)_k) (act-lo correction)
    A+C on all k-blocks; B on x and h; topic gets (A_t, B_t) in one
    DoubleRow pair. Measured rel err ~8e-3 (vs 1.9e-4 fp16 baseline).
  - Biases ride a constant-one 65th partition row of the topic pair, so the
    gate activations need no per-half bias reads and process both gate
    halves (2 PSUM banks) in one wide instruction.
  - Everything downstream of PSUM is fp16 (packed SBUF fp16 runs the DVE in
    its 4x perf mode); c is shipped fp16 for the elementwise cc = cf*c.
  - All per-chunk DMA is consolidated: host pre-tiles inputs into
    per-chunk-contiguous planes (one ~2us fixed cost per DMA instruction in
    the TRN2 model makes many small DMAs expensive), outputs land in a
    chunk-tiled fp16 layout un-tiled on the host.
"""

import os
import numpy as np
import ml_dtypes

import concourse.bass as bass
import concourse.bacc as bacc
import concourse.mybir as mybir
from concourse.tile import TileContext
from concourse.bass_utils import run_bass_kernel_spmd

I, H, T, B = 256, 256, 64, 65536
NCORES = 8
BS = B // NCORES          # 8192 batch columns per core
NT = 512                  # columns per outer chunk (one full PSUM bank of fp32)
NH = 256                  # columns per DoubleRow pass (rhs free cap is 2*NH=512)
NCHUNK = BS // NT         # 16

FP8 = mybir.dt.float8e4
FP16 = mybir.dt.float16
FP32 = mybir.dt.float32
SIG = mybir.ActivationFunctionType.Sigmoid
TANH = mybir.ActivationFunctionType.Tanh
DR = mybir.MatmulPerfMode.DoubleRow
E4NP = ml_dtypes.float8_e4m3

WSCALE = 16.0

# Weight-pair stack, in emission order. Entries: (kind, input) where kind
# selects WH16/WL16/WRAW and input selects the z pair slots in the fused
# per-chunk z tile: x=(0,1) h=(2,3) c=(4,5) lx=(6,7) lh=(8,9).
#   A = WH16 @ zh, C = WL16 @ zh, B = WRAW @ zl16
PAIRS = [
    ("A", "x"), ("A", "h"), ("A", "c"),
    ("C", "x"), ("C", "h"), ("C", "c"),
    ("B", "x"), ("B", "h"),
]
NPAIR = len(PAIRS)
ZSLOT = {"x": 0, "h": 2, "c": 4, "lx": 6, "lh": 8}
NZSLOT = 10
# pairs skipped for the candidate-gate m-blocks (no c input term)
SKIP_C = {i for i, (_, src) in enumerate(PAIRS) if src == "c"}
# act-lo (B) compensation only pays off on the candidate gate: its tanh has
# unit derivative and feeds cc directly, so it dominates the act-quantization
# error; the sigmoid gates are damped enough to skip (measured ladder:
# B on all gates 8.2e-3, B on candidate only ~1.4e-2, no B 1.9e-2).
B_PAIRS = {i for i, (kind, _) in enumerate(PAIRS) if kind == "B"}


def gate_pairs(g):
    return [p for p in range(NPAIR)
            if not (g == 2 and p in SKIP_C) and not (g != 2 and p in B_PAIRS)]
GATE_FN = [SIG, SIG, TANH, SIG]   # i, f, c~, o

_PROGRAM = None
_LAST_RESULTS = None  # for test harness introspection


def _build_program():
    nc = bacc.Bacc()

    # fused fp8 z plane: per chunk 10 slots x 512 cols per partition
    zall = nc.declare_dram_parameter("zall", [128, NCHUNK * NZSLOT * NT], FP8,
                                     isOutput=False)
    # topic pair plane (+ constant-one bias row at partition 64)
    ztp = nc.declare_dram_parameter("ztp", [65, NCHUNK * 2 * NT], FP8,
                                    isOutput=False)
    c16 = nc.declare_dram_parameter("c16", [128, NCHUNK * 2 * NT], FP16,
                                    isOutput=False)
    wt = nc.declare_dram_parameter("wt", [128, NPAIR * 2048], FP8, isOutput=False)
    wtt = nc.declare_dram_parameter("wtt", [65, 2048], FP8, isOutput=False)
    ch_out = nc.declare_dram_parameter("ch", [128, NCHUNK * 2 * NT], FP16,
                                       isOutput=True)
    cc_out = nc.declare_dram_parameter("cc", [128, NCHUNK * 2 * NT], FP16,
                                       isOutput=True)

    with TileContext(nc) as tc:
        with (
            tc.tile_pool(name="const", bufs=1) as constp,
            tc.tile_pool(name="zin", bufs=3) as zp,
            tc.tile_pool(name="gates", bufs=2) as gp,
            tc.tile_pool(name="psum", bufs=1, space="PSUM") as pp,
        ):
            wsb = [constp.tile([128, 2, 1024], FP8, tag=f"w{p}", name=f"wsb{p}")
                   for p in range(NPAIR)]
            wst = constp.tile([65, 2, 1024], FP8, tag="wt", name="wst")
            # first pair's weights immediately; the rest interleaved between
            # the first chunks' input DMAs (HWDGE queues run in parallel)
            nc.scalar.dma_start(out=wsb[0][:], in_=wt[:, 0:2048])
            nc.scalar.dma_start(out=wst[:], in_=wtt[:])
            for p in range(1, 6):
                nc.scalar.dma_start(out=wsb[p][:],
                                    in_=wt[:, p * 2048:(p + 1) * 2048])

            # PE warm-up: ~2.7us of tiny matmuls hidden under the initial DMA
            # fill releases the p-state ramp (cold PE runs slow for its first
            # ~3us of activity) before the real stream begins.
            wz = constp.tile([128, 64], FP16, tag="wz", name="wz")
            nc.vector.memset(wz[:], 0.0)
            # warm-up shares the gate-0 PSUM banks; pool deps order it first
            pdum = pp.tile([128, 2, NT], FP32, tag="pg0", name="pdum")
            for _ in range(28):
                nc.tensor.matmul(pdum[0:64, 0, 0:64], wz[:, 0:64], wz[:, 0:64],
                                 start=True, stop=True)

            for n in range(NCHUNK):
                # ---- one consolidated DMA per input plane per chunk ----
                zt = zp.tile([128, NZSLOT, NT], FP8, tag="z", name=f"z_{n}")
                z0 = n * NZSLOT * NT
                tt = zp.tile([65, 2, NT], FP8, tag="t", name=f"t_{n}")
                ct = zp.tile([128, 2, NT], FP16, tag="c", name=f"c_{n}")
                if n == 0:
                    # stage the first chunk so each DMA lands just before the
                    # phase-ordered matmuls need it
                    nc.sync.dma_start(out=zt[:, 0:2, :], in_=zall[:, z0:z0 + 2 * NT])
                    nc.sync.dma_start(out=tt[:], in_=ztp[:, 0:2 * NT])
                    nc.sync.dma_start(out=zt[:, 2:7, :],
                                      in_=zall[:, z0 + 2 * NT:z0 + 7 * NT])
                    nc.gpsimd.dma_start(out=zt[:, 7:NZSLOT, :],
                                        in_=zall[:, z0 + 7 * NT:z0 + NZSLOT * NT])
                    nc.gpsimd.dma_start(out=ct[:], in_=c16[:, 0:2 * NT])
                    for p in (6, 7):
                        nc.gpsimd.dma_start(out=wsb[p][:],
                                            in_=wt[:, p * 2048:(p + 1) * 2048])
                else:
                    nc.sync.dma_start(out=zt[:], in_=zall[:, z0:z0 + NZSLOT * NT])
                    nc.sync.dma_start(
                        out=tt[:], in_=ztp[:, n * 2 * NT:(n + 1) * 2 * NT])
                    nc.gpsimd.dma_start(
                        out=ct[:], in_=c16[:, n * 2 * NT:(n + 1) * 2 * NT])

                # ---- fused gate matmul: DoubleRow pairs ----
                # Gate g owns a [128, 2, 512] PSUM tile = one bank per gate
                # half; each (half, colh) quarter is one DoubleRow pass.
                # Chunk 0 is emitted pair-major (all A_x, topic, A_h, ...) to
                # match weight/z DMA arrival; later chunks gate-major so each
                # gate finishes early for its activation.
                pg = [pp.tile([128, 2, NT], FP32, tag=f"pg{g}", name=f"pg{g}_{n}")
                      for g in range(4)]
                TOPIC = -1
                ops = []  # (pair or TOPIC, g, hf, colh)
                if n == 0:
                    order = [0, TOPIC, 1, 2, 3, 4, 5, 6, 7]
                    for p in order:
                        for g in range(4):
                            if p != TOPIC and p not in gate_pairs(g):
                                continue
                            for hf in range(2):
                                for colh in range(2):
                                    ops.append((p, g, hf, colh))
                else:
                    for g in range(4):
                        for hf in range(2):
                            for colh in range(2):
                                for p in gate_pairs(g) + [TOPIC]:
                                    ops.append((p, g, hf, colh))
                started = set()
                last_op = {}
                for k, (p, g, hf, colh) in enumerate(ops):
                    last_op[(g, hf)] = k
                for k, (p, g, hf, colh) in enumerate(ops):
                    m = 2 * g + hf
                    ms, me = m * 128, (m + 1) * 128
                    cs = slice(colh * NH, (colh + 1) * NH)
                    if p == TOPIC:
                        lhsT, rhs = wst[:, :, ms:me], tt[:, :, cs]
                    else:
                        kind, src = PAIRS[p]
                        zs = ZSLOT[src if kind != "B" else "l" + src]
                        lhsT, rhs = wsb[p][:, :, ms:me], zt[:, zs:zs + 2, cs]
                    key = (g, hf)
                    nc.tensor.matmul(pg[g][:, hf, cs], lhsT, rhs,
                                     start=(key not in started),
                                     stop=(last_op[key] == k), perf_mode=DR)
                    started.add(key)

                # ---- wide gate activations (x16 weight scale undone here) ----
                def act(g, nm, in_=None):
                    t = gp.tile([128, 2, NT], FP16, tag=nm, name=f"{nm}_{n}")
                    nc.scalar.activation(out=t[:], in_=in_ if in_ is not None
                                         else pg[g][:], func=GATE_FN[g] if in_ is None else TANH,
                                         scale=1.0 / WSCALE if in_ is None else 1.0)
                    return t

                last = n == NCHUNK - 1
                ci = act(0, "ci")
                cf = act(1, "cf")
                tg = act(2, "tg")
                if not last:
                    co = act(3, "co")
                else:
                    # per-half drain shortens the final dependency chain
                    co = gp.tile([128, 2, NT], FP16, tag="co", name=f"co_{n}")
                    for hf in range(2):
                        nc.scalar.activation(out=co[:, hf, :], in_=pg[3][:, hf, :],
                                             func=SIG, scale=1.0 / WSCALE)

                # ---- fp16 elementwise (DVE 4x perf mode) ----
                t1 = gp.tile([128, 2, NT], FP16, tag="t1", name=f"t1_{n}")
                nc.vector.tensor_mul(t1[:], ci[:], tg[:])
                t2 = gp.tile([128, 2, NT], FP16, tag="t2", name=f"t2_{n}")
                nc.vector.tensor_mul(t2[:], cf[:], ct[:])
                cct = gp.tile([128, 2, NT], FP16, tag="cc", name=f"cc_{n}")
                nc.vector.tensor_add(cct[:], t1[:], t2[:])
                tcc = act(None, "tcc", in_=cct)
                cht = gp.tile([128, 2, NT], FP16, tag="chh", name=f"chh_{n}")
                ob = n * 2 * NT
                nc.gpsimd.dma_start(out=cc_out[:, ob:ob + 2 * NT], in_=cct[:])
                if not last:
                    nc.vector.tensor_mul(cht[:], co[:], tcc[:])
                    nc.gpsimd.dma_start(out=ch_out[:, ob:ob + 2 * NT], in_=cht[:])
                else:
                    for hf in range(2):
                        nc.vector.tensor_mul(cht[:, hf, :], co[:, hf, :],
                                             tcc[:, hf, :])
                        eng = nc.sync if hf == 0 else nc.gpsimd
                        eng.dma_start(out=ch_out[:, ob + hf * NT:ob + (hf + 1) * NT],
                                      in_=cht[:, hf, :])

    nc.finalize()
    return nc


def _q8(a):
    return a.astype(E4NP)


def _prep_weights(inp):
    """Fused (1024, 832) weights -> hi16/lo16/raw fp8 pair stacks."""
    Wf = np.zeros((1024, 832), np.float32)

    def put(g, blocks):
        r = g * 256
        for j, wb in enumerate(blocks):
            if wb is None:
                continue
            col = j * 256
            Wf[r:r + 256, col:col + wb.shape[1]] = wb

    put(0, [inp["w_ii"], inp["w_hi"], inp["w_ci"], inp["w_bi"]])
    put(1, [inp["w_if"], inp["w_hf"], inp["w_cf"], inp["w_bf"]])
    put(2, [inp["w_ic"], inp["w_hc"], None, inp["w_bc"]])
    put(3, [inp["w_io"], -inp["w_ho"], inp["w_co"], inp["w_bo"]])

    wT = Wf.T  # [832, 1024] k-major
    wh16 = _q8(WSCALE * wT)
    wl16 = _q8(WSCALE * wT - wh16.astype(np.float32))
    wraw = _q8(wT)

    kblk = {"x": (0, 128), "h": (256, 384), "c": (512, 640)}
    stacks = {"A": wh16, "C": wl16, "B": wraw}
    wt_host = np.zeros((128, NPAIR, 2, 1024), E4NP)
    for p, (kind, src) in enumerate(PAIRS):
        r0, r1 = kblk[src]
        wt_host[:, p, 0, :] = stacks[kind][r0:r0 + 128]
        wt_host[:, p, 1, :] = stacks[kind][r1:r1 + 128]
    wt_host = np.ascontiguousarray(wt_host.reshape(128, NPAIR * 2048))

    bias_vec = np.concatenate(
        [inp["bias_i"], inp["bias_f"], inp["bias_c"], inp["bias_o"]],
        axis=0).reshape(1024)
    wtt_host = np.zeros((65, 2, 1024), E4NP)
    wtt_host[:64, 0, :] = wh16[768:832]           # A_t
    wtt_host[:64, 1, :] = wraw[768:832]           # B_t
    wtt_host[64, 0, :] = _q8(WSCALE * bias_vec)   # bias rides slot 0
    wtt_host = np.ascontiguousarray(wtt_host.reshape(65, 2048))
    return wt_host, wtt_host


def _chunk_tile(a):
    """[R, BS] -> [R, NCHUNK, BS//NCHUNK] view of per-chunk columns."""
    return a.reshape(a.shape[0], NCHUNK, NT)


def kernel(**inputs):
    global _PROGRAM, _LAST_RESULTS
    if _PROGRAM is None:
        _PROGRAM = _build_program()
    nc = _PROGRAM

    inp = {k: np.asarray(v, dtype=np.float32) for k, v in inputs.items()}
    wt_host, wtt_host = _prep_weights(inp)

    zfull = np.concatenate(
        [inp["x"], inp["h"], inp["c"], inp["topic"]], axis=0)  # [832, B]
    zhi_all = _q8(zfull)
    res16 = _q8(WSCALE * (zfull - zhi_all.astype(np.float32)))
    c16_all = inp["c"].astype(np.float16)

    in_maps = []
    for i in range(NCORES):
        sl = slice(i * BS, (i + 1) * BS)
        zhi = zhi_all[:, sl]
        zlo = res16[:, sl]
        # z slots: x0 x1 h0 h1 c0 c1 | lx0 lx1 lh0 lh1
        slots = [zhi[r:r + 128] for r in range(0, 768, 128)] + \
                [zlo[r:r + 128] for r in range(0, 512, 128)]
        za = np.stack([_chunk_tile(s) for s in slots], axis=2)  # [128,NCHUNK,10,512]
        za = np.ascontiguousarray(za.reshape(128, NCHUNK * NZSLOT * NT))

        tp = np.empty((65, NCHUNK, 2, NT), E4NP)
        tp[:64, :, 0, :] = _chunk_tile(zhi[768:832])
        tp[:64, :, 1, :] = _chunk_tile(zlo[768:832])
        tp[64, :, 0, :] = np.float32(1.0)
        tp[64, :, 1, :] = np.float32(0.0)
        tp = np.ascontiguousarray(tp.reshape(65, NCHUNK * 2 * NT))

        cfull = c16_all[:, sl]
        cm = np.stack([_chunk_tile(cfull[0:128]), _chunk_tile(cfull[128:256])],
                      axis=2)  # [128, NCHUNK, 2, 512]
        cm = np.ascontiguousarray(cm.reshape(128, NCHUNK * 2 * NT))

        in_maps.append({
            "zall": za, "ztp": tp, "c16": cm,
            "wt": wt_host, "wtt": wtt_host,
        })

    res = run_bass_kernel_spmd(
        nc, in_maps, list(range(NCORES)),
        trace=bool(os.environ.get("KERNEL_TRACE")),
    )
    _LAST_RESULTS = res

    def untile(name):
        parts = []
        for i in range(NCORES):
            a = res.results[i][name].astype(np.float32)
            a = a.reshape(128, NCHUNK, 2, NT).transpose(2, 0, 1, 3)
            parts.append(a.reshape(256, BS))
        return np.concatenate(parts, axis=1)

    return np.stack([untile("ch"), untile("cc")], axis=0)
